# revision 19
# baseline (speedup 1.0000x reference)
"""Trainium2 Bass kernel for nn_Block_73967926771745 (dense transformer block).

Sharding over 8 NeuronCores:
  - Attention: head-parallel (1 head/core, all 4 batches), AllGather heads.
  - Conv MLP: (4 Cout-blocks x 2 batch-halves) grid; every conv matmul has
    M=128. AllGathers in groups [[0-3],[4-7]] (a group shares one batch-half,
    so conv programs are fully static). Convs = sums of shifted 1x1 matmuls
    over padded channel-major layouts, bf16 x bf16 -> fp32 PSUM.
  - The only per-core dynamic indexing is at the attention->conv handoff
    (bsel/csel scalar inputs select the core's 2 batches / channel block).
"""

import numpy as np
import ml_dtypes

import concourse.bass as bass
import concourse.bacc as bacc
import concourse.tile as tile
import concourse.mybir as mybir
from concourse import bass_utils
from concourse.masks import make_identity

F32 = mybir.dt.float32
BF16 = mybir.dt.bfloat16
AF = mybir.ActivationFunctionType
ALU = mybir.AluOpType

B, Hh, Ww, C, HEADS, HID = 4, 28, 28, 256, 8, 512
N = Hh * Ww            # 784
HD = C // HEADS        # 32
EPS = 1e-5
NCORES = 8
NB = 2                 # local batches per core in conv stage
PTOK = 112             # token-tile partition size (784 = 7*112)
NTT = 7                # token tiles per batch
NTILES = B * NTT       # 28
CHK = 392              # free-dim chunk = 14 rows of 28
NCHK = NB * 2          # conv chunks (2 batches x 2 row-halves)

# pp (per-partition param table) column indices
PP_QB, PP_KB, PP_C1B, PP_C1S, PP_C1T = 0, 1, 2, 3, 4
PP_AS = {0: 5, 1: 7, 2: 9}     # a-conv bn scale (shift = +1)
PP_BS = {0: 11, 1: 13, 2: 15}  # b-conv bn scale (shift = +1)
PP_CATS, PP_RESS, PP_CATRT = 17, 18, 19
PP_PBNS, PP_PBNT = 20, 21
PP_P2S, PP_P2T = 22, 23
PP_DWB = 24
PP_DWW = 25            # 25..33 (9 taps)
PP_G2 = {0: 34, 1: 35}
PP_B2 = {0: 36, 1: 37}
NPP = 38

ACONVS = [(3, 1, "wb1a"), (5, 2, "wb2a"), (7, 3, "wb3a")]  # (k, pad, name)
BCONVS = [(3, "wb1b"), (5, "wb2b"), (7, "wb3b")]           # (dilation, name)

# set to a stage name to truncate the program and emit a debug output
DEBUG_STAGE = None
# swapped to a sim-supported function for cost-model simulation runs
GELU = AF.Gelu


def _build(debug_stage=None, reps=1):
    nc = bacc.Bacc("TRN2", target_bir_lowering=False, debug=False,
                   num_devices=NCORES)

    # ---------------- I/O declarations ----------------
    t = {}
    t["x_in"] = nc.dram_tensor("x_in", [B, N, C], F32, kind="ExternalInput")
    t["rpT_in"] = nc.dram_tensor("rpT_in", [N, N], BF16, kind="ExternalInput")
    t["bsel_in"] = nc.dram_tensor("bsel_in", [1, 1], mybir.dt.uint32,
                                  kind="ExternalInput")
    t["csel_in"] = nc.dram_tensor("csel_in", [1, 1], mybir.dt.uint32,
                                  kind="ExternalInput")
    for nm, sh in [("wq_in", [2, 128, HD]), ("wk_in", [2, 128, HD]),
                   ("wv_in", [2, 128, HD]), ("wp_in", [2, 128, C]),
                   ("wc1_in", [2, 128, 128]), ("wres_in", [4, 128, 128]),
                   ("wp2_in", [4, 128, 64]), ("wcat_in", [12, 128, 9 * 128])]:
        t[nm] = nc.dram_tensor(nm, sh, BF16, kind="ExternalInput")
    for k, pad, name in ACONVS:
        t[name] = nc.dram_tensor(name, [4, 128, k * k * 128], BF16,
                                 kind="ExternalInput")
    for d, name in BCONVS:
        t[name] = nc.dram_tensor(name, [4, 128, 9 * 128], BF16,
                                 kind="ExternalInput")
    t["bv_in"] = nc.dram_tensor("bv_in", [1, HD], F32, kind="ExternalInput")
    t["bp_in"] = nc.dram_tensor("bp_in", [1, C], F32, kind="ExternalInput")
    t["tz_in"] = nc.dram_tensor("tz_in", [7, 68, 28], F32, kind="ExternalInput")
    t["pp_in"] = nc.dram_tensor("pp_in", [128, NPP], F32, kind="ExternalInput")
    t["zeros_in"] = nc.dram_tensor("zeros_in", [128, 3600], BF16,
                                   kind="ExternalInput")

    t["out_d"] = nc.dram_tensor("out", [NB, N, 64], F32, kind="ExternalOutput")
    t["dbg_d"] = None
    if debug_stage is not None:
        t["dbg_d"] = nc.dram_tensor("dbg", [128, 8192], F32,
                                    kind="ExternalOutput")

    # collective DRAM buffers
    t["ag_o_in"] = nc.dram_tensor("ag_o_in", [HD, B * N], BF16, kind="Internal")
    t["ag_o_out"] = nc.dram_tensor("ag_o_out", [C, B * N], BF16,
                                   kind="Internal", addr_space="Shared")
    ag_cols = {"agf1": NB * 34 * 34, "aga0": NB * 34 * 34,
               "aga1": NB * 38 * 38, "aga2": NB * 42 * 42,
               "agb0": NB * 30 * 30, "agb1": NB * 30 * 30,
               "agb2": NB * 30 * 30, "agg": NB * N}
    for nm, cols in ag_cols.items():
        t[nm + "_in"] = nc.dram_tensor(nm + "_in", [128, cols], BF16,
                                       kind="Internal")
        t[nm + "_out"] = nc.dram_tensor(nm + "_out", [1024, cols], BF16,
                                        kind="Internal", addr_space="Shared")
    t["G4"] = [[0, 1, 2, 3], [4, 5, 6, 7]]
    t["G8"] = [[0, 1, 2, 3, 4, 5, 6, 7]]

    with tile.TileContext(nc) as tc:
        for rep in range(reps):
            _body(nc, tc, t, debug_stage, rep)
    nc.compile()
    return nc


class _Trunc(Exception):
    pass


def _body(nc, tc, t, debug_stage, rep=0):
    pools = []
    try:
        _body_inner(nc, tc, t, debug_stage, rep, pools)
    except _Trunc:
        pass
    for pool in reversed(pools):
        if not pool._released:
            pool.release()


def _body_inner(nc, tc, t, debug_stage, rep, pools):
    # ---------------- persistent pools ----------------
    consts = tc.alloc_tile_pool(name="consts", bufs=1)
    pools.append(consts)
    pp = consts.tile([128, NPP], F32)
    nc.sync.dma_start(pp[:], t["pp_in"][:])
    ident_f = consts.tile([128, 128], F32)
    make_identity(nc, ident_f)
    ident_b = consts.tile([128, 128], BF16)
    make_identity(nc, ident_b)
    ones_f = consts.tile([1, 128], F32)
    nc.vector.memset(ones_f[:], 1.0)
    ones_bcol = consts.tile([128, 1], BF16)
    nc.vector.memset(ones_bcol[:], 1.0)
    ones_brow = consts.tile([1, 128], BF16)
    nc.vector.memset(ones_brow[:], 1.0)
    bv_bc = consts.tile([128, HD], F32)
    nc.sync.dma_start(bv_bc[:], t["bv_in"].ap().partition_broadcast(128))
    bp_bc = consts.tile([128, C], F32)
    nc.sync.dma_start(bp_bc[:], t["bp_in"].ap().partition_broadcast(128))
    eps_t = consts.tile([128, 1], F32)
    nc.vector.memset(eps_t[:], EPS)

    def dbg_emit(stage, aps):
        """DMA the given SBUF APs (flattened side by side) to debug output."""
        if debug_stage != stage:
            return False
        col = 0
        for ap in aps:
            p, f = ap.partition_size(), ap.free_size()
            if ap.dtype != F32:
                tmp = consts.tile([128, f], F32, tag=f"dbg{col}")
                nc.vector.tensor_copy(tmp[0:p, :], ap)
                ap = tmp[0:p, :]
            nc.sync.dma_start(t["dbg_d"][0:p, col:col + f], ap)
            col += f
        raise _Trunc()

    # dynamic selectors
    breg = nc.alloc_registers(f"bsel_reg{rep}")
    nc.regs_load(breg, t["bsel_in"][0:1, 0:1])
    bsel = nc.snap(breg, donate=True, min_val=0, max_val=2)
    creg = nc.alloc_registers(f"csel_reg{rep}")
    nc.regs_load(creg, t["csel_in"][0:1, 0:1])
    csel = nc.snap(creg, donate=True, min_val=0, max_val=192)

    def emit_ag(nm, src_ap):
        nc.sync.dma_start(t[nm + "_in"].ap(), src_ap)
        nc.gpsimd.collective_compute(
            "AllGather", ALU.bypass, replica_groups=t["G8"],
            ins=[t[nm + "_in"].ap()], outs=[t[nm + "_out"].ap()])

    def ag_rows(nm, ci):
        return t[nm + "_out"][bass.ds(bsel * 256 + ci * 128, 128), :]

    # persistent activations
    persist = tc.alloc_tile_pool(name="persist", bufs=1)
    pools.append(persist)
    fg = [persist.tile([128, B, N], BF16, tag=f"fg{i}", name=f"fg{i}") for i in range(2)]
    x1loc = persist.tile([PTOK, NB * NTT, 64], F32, tag="x1loc")
    ftp = tc.alloc_tile_pool(name="ftp", bufs=1)
    pools.append(ftp)
    fT = [ftp.tile([128, B, N], BF16, tag=f"fT{i}", name=f"fT{i}") for i in range(2)]

    # =====================================================================
    # Phase 1: attention
    # =====================================================================
    with tc.tile_pool(name="attn", bufs=2) as attn, \
         tc.tile_pool(name="attn1", bufs=1) as attn1, \
         tc.tile_pool(name="ps_s", bufs=2, space="PSUM") as ps_s, \
         tc.tile_pool(name="ps_o", bufs=1, space="PSUM") as ps_o, \
         tc.tile_pool(name="ps_r", bufs=1, space="PSUM") as ps_r, \
         tc.tile_pool(name="ps_m", bufs=2, space="PSUM") as ps_m:

        x1_all = attn1.tile([PTOK, NTILES, C], F32, tag="x1")
        # weights
        wq_sb = attn1.tile([128, 2, HD], BF16, tag="wq")
        wk_sb = attn1.tile([128, 2, HD], BF16, tag="wk")
        wv_sb = attn1.tile([128, 2, HD], BF16, tag="wv")
        wp_sb = attn1.tile([128, 2, C], BF16, tag="wp")
        for c2 in range(2):
            nc.sync.dma_start(wq_sb[:, c2, :], t["wq_in"][c2])
            nc.sync.dma_start(wk_sb[:, c2, :], t["wk_in"][c2])
            nc.sync.dma_start(wv_sb[:, c2, :], t["wv_in"][c2])
            nc.sync.dma_start(wp_sb[:, c2, :], t["wp_in"][c2])
        rp_sb = attn1.tile([PTOK, NTT, N], BF16, tag="rp")
        for j in range(NTT):
            nc.sync.dma_start(rp_sb[:, j, :],
                              t["rpT_in"][j * PTOK:(j + 1) * PTOK, :])

        # ---- LayerNorm1 (token-major) -> z (bf16); keep x tiles ----
        x_sb = attn1.tile([PTOK, NTILES, C], F32, tag="x")
        z_sb = attn1.tile([PTOK, NTILES, C], BF16, tag="z")
        for ti in range(NTILES):
            b, j = divmod(ti, NTT)
            nc.sync.dma_start(x_sb[:, ti, :],
                              t["x_in"][b, j * PTOK:(j + 1) * PTOK, :])
        for ti in range(NTILES):
            stats = attn.tile([PTOK, 6], F32, tag="stats")
            mv = attn.tile([PTOK, 2], F32, tag="mv")
            nc.vector.bn_stats(stats[:], x_sb[:, ti, :])
            nc.vector.bn_aggr(mv[:], stats[:])
            rstd = attn.tile([PTOK, 1], F32, tag="rstd")
            nc.scalar.activation(rstd[:], mv[:, 1:2], AF.Sqrt, bias=eps_t[0:PTOK])
            nc.vector.reciprocal(rstd[:], rstd[:])
            nc.vector.tensor_scalar(z_sb[:, ti, :], x_sb[:, ti, :],
                                    mv[:, 0:1], rstd[:], ALU.subtract, ALU.mult)
        if dbg_emit("ln1", [z_sb[:, ti, :] for ti in range(4)]):
            return

        # ---- zT (channel-major bf16) via PE transpose ----
        zT = attn1.tile([128, 2, B, N], BF16, tag="zT")
        for ti in range(NTILES):
            b, j = divmod(ti, NTT)
            for c2 in range(2):
                tp = ps_m.tile([128, PTOK], BF16, tag="pm")
                nc.tensor.transpose(tp[:], z_sb[:, ti, c2 * 128:(c2 + 1) * 128],
                                    ident_b[0:PTOK, 0:PTOK])
                nc.scalar.copy(zT[:, c2, b, j * PTOK:(j + 1) * PTOK], tp[:])

        # ---- q/k (channel-major [HD, N]) and v (token-major, +ones col) ----
        qT = attn1.tile([HD, B, N], BF16, tag="qT")
        kT = attn1.tile([HD, B, N], BF16, tag="kT")
        v_sb = attn1.tile([PTOK, B, NTT, HD + 1], BF16, tag="v")
        for b in range(B):
            for qc in range(2):
                sl = slice(qc * CHK, (qc + 1) * CHK)
                pq = ps_m.tile([HD, CHK], F32, tag="pm")
                for c2 in range(2):
                    nc.tensor.matmul(pq[:], wq_sb[:, c2, :], zT[:, c2, b, sl],
                                     start=(c2 == 0), stop=(c2 == 1))
                nc.vector.tensor_scalar(qT[:, b, sl], pq[:],
                                        pp[0:HD, PP_QB:PP_QB + 1], None, ALU.add)
                pk = ps_m.tile([HD, CHK], F32, tag="pm")
                for c2 in range(2):
                    nc.tensor.matmul(pk[:], wk_sb[:, c2, :], zT[:, c2, b, sl],
                                     start=(c2 == 0), stop=(c2 == 1))
                nc.vector.tensor_scalar(kT[:, b, sl], pk[:],
                                        pp[0:HD, PP_KB:PP_KB + 1], None, ALU.add)
            for j in range(NTT):
                pv = ps_m.tile([PTOK, HD], F32, tag="pm")
                for c2 in range(2):
                    nc.tensor.matmul(pv[:],
                                     zT[:, c2, b, j * PTOK:(j + 1) * PTOK],
                                     wv_sb[:, c2, :],
                                     start=(c2 == 0), stop=(c2 == 1))
                nc.vector.tensor_tensor(v_sb[:, b, j, 0:HD], pv[:],
                                        bv_bc[0:PTOK], ALU.add)
                nc.vector.memset(v_sb[:, b, j, HD:HD + 1], 1.0)
        if dbg_emit("qkv", [qT[:, 0, :], kT[:, 0, :],
                            v_sb[:, 0, :, :].rearrange("p a b -> p (a b)")]):
            return

        # ---- scores^T -> +rp -> exp -> PV -> normalize ----
        o_sb = attn1.tile([HD, B, N], BF16, tag="o")
        for b in range(B):
            es = attn.tile([PTOK, NTT, N], BF16, tag="es")
            for j in range(NTT):
                tsl = slice(j * PTOK, (j + 1) * PTOK)
                for qc in range(2):
                    sl = slice(qc * CHK, (qc + 1) * CHK)
                    ps = ps_s.tile([PTOK, CHK], F32, tag="ps")
                    nc.tensor.matmul(ps[:], kT[:, b, tsl], qT[:, b, sl],
                                     start=True, stop=True)
                    st = attn.tile([PTOK, CHK], BF16, tag="st")
                    nc.vector.tensor_tensor(st[:], ps[:], rp_sb[:, j, sl], ALU.add)
                    nc.scalar.activation(es[:, j, sl], st[:], AF.Exp)
            for qc in range(2):
                sl = slice(qc * CHK, (qc + 1) * CHK)
                po = ps_o.tile([HD + 1, CHK], F32, tag="po")
                for j in range(NTT):
                    nc.tensor.matmul(po[:], v_sb[:, b, j, :], es[:, j, sl],
                                     start=(j == 0), stop=(j == NTT - 1))
                inv = attn.tile([1, CHK], F32, tag="inv")
                nc.vector.reciprocal(inv[:], po[HD:HD + 1, :])
                pr = ps_r.tile([HD, CHK], F32, tag="pr")
                nc.tensor.matmul(pr[:], ones_f[0:1, 0:HD], inv[:],
                                 start=True, stop=True)
                rep = attn.tile([HD, CHK], F32, tag="rep")
                nc.scalar.copy(rep[:], pr[:])
                nc.vector.tensor_tensor(o_sb[:, b, sl], po[0:HD, :], rep[:],
                                        ALU.mult)
        if dbg_emit("oattn", [o_sb[:, b, :] for b in range(B)]):
            return

        # ---- AllGather heads ----
        nc.sync.dma_start(t["ag_o_in"].ap(),
                          o_sb[:].rearrange("p b n -> p (b n)"))
        nc.gpsimd.collective_compute(
            "AllGather", ALU.bypass, replica_groups=t["G8"],
            ins=[t["ag_o_in"].ap()], outs=[t["ag_o_out"].ap()])
        of = attn1.tile([128, 2, B * N], BF16, tag="of")
        for c2 in range(2):
            nc.sync.dma_start(of[:, c2, :],
                              t["ag_o_out"][c2 * 128:(c2 + 1) * 128, :])

        # ---- out proj + residual -> x1 (token-major fp32) ----
        for ti in range(NTILES):
            tsl = slice(ti * PTOK, (ti + 1) * PTOK)
            pj = ps_s.tile([PTOK, C], F32, tag="ps")
            for c2 in range(2):
                nc.tensor.matmul(pj[:], of[:, c2, tsl], wp_sb[:, c2, :],
                                 start=(c2 == 0), stop=(c2 == 1))
            nc.vector.tensor_tensor(x1_all[:, ti, :], pj[:], x_sb[:, ti, :],
                                    ALU.add)
            nc.vector.tensor_tensor(x1_all[:, ti, :], x1_all[:, ti, :],
                                    bp_bc[0:PTOK], ALU.add)
        if dbg_emit("x1", [x1_all[:, ti, :] for ti in range(8)]):
            return

        # ---- LayerNorm2 -> z2 -> transpose + ln2 affine -> fT ----
        for ti in range(NTILES):
            stats = attn.tile([PTOK, 6], F32, tag="stats")
            mv = attn.tile([PTOK, 2], F32, tag="mv")
            nc.vector.bn_stats(stats[:], x1_all[:, ti, :])
            nc.vector.bn_aggr(mv[:], stats[:])
            rstd = attn.tile([PTOK, 1], F32, tag="rstd")
            nc.scalar.activation(rstd[:], mv[:, 1:2], AF.Sqrt, bias=eps_t[0:PTOK])
            nc.vector.reciprocal(rstd[:], rstd[:])
            nc.vector.tensor_scalar(z_sb[:, ti, :], x1_all[:, ti, :],
                                    mv[:, 0:1], rstd[:], ALU.subtract, ALU.mult)
        for ti in range(NTILES):
            b, j = divmod(ti, NTT)
            for c2 in range(2):
                tp = ps_m.tile([128, PTOK], BF16, tag="pm")
                nc.tensor.transpose(tp[:], z_sb[:, ti, c2 * 128:(c2 + 1) * 128],
                                    ident_b[0:PTOK, 0:PTOK])
                nc.vector.tensor_scalar(fT[c2][:, b, j * PTOK:(j + 1) * PTOK],
                                        tp[:],
                                        pp[:, PP_G2[c2]:PP_G2[c2] + 1],
                                        pp[:, PP_B2[c2]:PP_B2[c2] + 1],
                                        ALU.mult, ALU.add)

        # ---- x1loc: this core's 2 batches, 64-ch slice (dynamic DMA) ----
        nc.sync.dma_start(
            x1loc[:],
            x1_all[:, bass.ds(bsel * NTT, NB * NTT), bass.ds(csel, 64)])
        if dbg_emit("fT", [fT[0][:, 0, :], fT[1][:, 0, :]]):
            return

    # =====================================================================
    # Phase 2: spatial-attention gate
    # =====================================================================
    with tc.tile_pool(name="sa", bufs=1) as sap, \
         tc.tile_pool(name="sps", bufs=2, space="PSUM") as sps:
        # mean over channels via ones-matmul
        meanr = sap.tile([1, B * N], F32, tag="meanr")
        for ch in range(8):
            sl = slice(ch * CHK, (ch + 1) * CHK)
            pm = sps.tile([1, CHK], F32, tag="pm")
            for c2 in range(2):
                fv = fT[c2][:].rearrange("p b n -> p (b n)")[:, sl]
                nc.tensor.matmul(pm[:], ones_bcol[:], fv,
                                 start=(c2 == 0), stop=(c2 == 1))
            nc.scalar.mul(meanr[:, sl], pm[:], 1.0 / C)
        # max over channels: pairwise tree with DMA partition-shifts
        mx = sap.tile([128, B * N], BF16, tag="mx")
        mxs = sap.tile([64, B * N], BF16, tag="mxs")
        nc.vector.tensor_max(mx[:], fT[0][:].rearrange("p b n -> p (b n)"),
                             fT[1][:].rearrange("p b n -> p (b n)"))
        w = 64
        while w >= 1:
            nc.sync.dma_start(mxs[0:w, :], mx[w:2 * w, :])
            nc.vector.tensor_max(mx[0:w, :], mx[0:w, :], mxs[0:w, :])
            w //= 2
        maxr = sap.tile([1, B * N], F32, tag="maxr")
        nc.vector.tensor_copy(maxr[:], mx[0:1, :])
        # stage into [ (c,ypad) 68, B, xpad 34 ] fp32, margins zero
        sain = sap.tile([68, B, 34], F32, tag="sain")
        nc.vector.memset(sain[:].rearrange("p a b -> p (a b)"), 0.0)
        for ci, srct in ((0, meanr), (1, maxr)):
            pstep = srct[:].ap[0][0]
            for b in range(B):
                src_r = bass.AP(tensor=srct.tensor,
                                offset=srct[:].offset + b * N,
                                ap=[[pstep, 1], [28, 28], [1, 28]])
                nc.sync.dma_start(sain[ci * 34 + 3:ci * 34 + 31, b, 3:31],
                                  src_r)
        # 7x7 conv (2ch->1) via 7 Toeplitz matmuls (y via lhsT, x via slicing)
        tz_sb = sap.tile([68, 7, 28], F32, tag="tz")
        nc.sync.dma_start(tz_sb[:], t["tz_in"].ap().transpose([1, 0, 2]))
        psa = sps.tile([28, B, 28], F32, tag="psa")
        for dx in range(7):
            nc.tensor.matmul(psa[:], tz_sb[:, dx, :], sain[:, :, dx:dx + 28],
                             start=(dx == 0), stop=(dx == 6))
        # 1 + sigmoid -> flatten to [1, B*N] -> replicate -> gate
        sig1 = sap.tile([28, B, 28], BF16, tag="sig1")
        nc.scalar.activation(sig1[:], psa[:], AF.Sigmoid)
        nc.vector.tensor_scalar(sig1[:], sig1[:], 1.0, None, ALU.add)
        onepsa = sap.tile([1, B * N], BF16, tag="onepsa")
        opstep = onepsa[:].ap[0][0]
        for b in range(B):
            dst_r = bass.AP(tensor=onepsa.tensor,
                            offset=onepsa[:].offset + b * N,
                            ap=[[opstep, 1], [28, 28], [1, 28]])
            nc.sync.dma_start(dst_r, sig1[:, b, :])
        if dbg_emit("sa", [meanr[:], maxr[:], onepsa[:]]):
            return
        for ch in range(8):
            sl = slice(ch * CHK, (ch + 1) * CHK)
            pg = sps.tile([128, CHK], F32, tag="pg")
            nc.tensor.matmul(pg[:], ones_brow[:], onepsa[:, sl],
                             start=True, stop=True)
            for c2 in range(2):
                nc.vector.tensor_tensor(
                    fg[c2][:].rearrange("p b n -> p (b n)")[:, sl], pg[:],
                    fT[c2][:].rearrange("p b n -> p (b n)")[:, sl], ALU.mult)
        if dbg_emit("fgate", [fg[0][:, 0, :], fg[1][:, 0, :]]):
            return

    ftp.release()
    # =====================================================================
    # Phase 3: conv stack (local 2 batches)
    # =====================================================================
    with tc.tile_pool(name="cv1", bufs=1) as cv1, \
         tc.tile_pool(name="cstream", bufs=2) as cstream, \
         tc.tile_pool(name="ctmp", bufs=3) as ctmp, \
         tc.tile_pool(name="cacc", bufs=1, space="PSUM") as cacc, \
         tc.tile_pool(name="cmisc", bufs=2, space="PSUM") as cmisc:

        # dynamic slice: this core's two batches of gated input
        fgl = cv1.tile([128, 2, NB * N], BF16, tag="fgl")
        for c2 in range(2):
            nc.sync.dma_start(
                fgl[:, c2, :],
                fg[c2][:].rearrange("p b n -> p (b n)")[:, bass.ds(bsel * N,
                                                                   NB * N)])

        # ---- c1 (1x1, 256->128) + gelu + BN -> f1 padded (pad 3) ----
        wc1_sb = cv1.tile([128, 2, 128], BF16, tag="wc1")
        for c2 in range(2):
            nc.sync.dma_start(wc1_sb[:, c2, :], t["wc1_in"][c2])
        f1p = [cv1.tile([128, NB, 34, 34], BF16, tag=f"f1p{i}", name=f"f1p{i}") for i in range(4)]
        for ci in range(4):
            nc.sync.dma_start(f1p[ci][:].rearrange("p b c d -> p (b c d)"),
                              t["zeros_in"][:, 0:NB * 34 * 34])
        for chk in range(NCHK):
            b, hf = divmod(chk, 2)
            y0 = hf * 14
            pc = cacc.tile([128, CHK], F32, tag="pacc0")
            for c2 in range(2):
                nc.tensor.matmul(
                    pc[:], wc1_sb[:, c2, :],
                    fgl[:, c2, b * N + y0 * 28: b * N + y0 * 28 + CHK],
                    start=(c2 == 0), stop=(c2 == 1))
            gl = ctmp.tile([128, CHK], F32, tag="gl")
            nc.scalar.activation(gl[:], pc[:], GELU,
                                 bias=pp[:, PP_C1B:PP_C1B + 1])
            nc.vector.tensor_scalar(f1p[0][:, b, 3 + y0:17 + y0, 3:31], gl[:],
                                    pp[:, PP_C1S:PP_C1S + 1],
                                    pp[:, PP_C1T:PP_C1T + 1], ALU.mult, ALU.add)
        # AG f1 (padded layout, fully contiguous transfers)
        emit_ag("agf1", f1p[0][:].rearrange("p b c d -> p (b c d)"))
        for ci in range(4):
            nc.sync.dma_start(f1p[ci][:].rearrange("p b c d -> p (b c d)"),
                              ag_rows("agf1", ci))
        if dbg_emit("f1", [f1p[ci][:].rearrange("p b c d -> p (b c d)")
                           for ci in range(2)]):
            return

        # ---- a-convs (3x3 / 5x5 / 7x7 on f1) ----
        for ic, (k, pad, wname) in enumerate(ACONVS):
            T = k * k
            Ws = 28 + 2 * (ic * 2 + 3)   # stage pad = b-conv dilation (3/5/7)
            ps_pad = ic * 2 + 3
            astg = cstream.tile([128, NB * 42 * 42], BF16, tag="astg")
            nc.sync.dma_start(astg[:, 0:NB * Ws * Ws],
                              t["zeros_in"][:, 0:NB * Ws * Ws])
            astg_v = astg[:, 0:NB * Ws * Ws].rearrange(
                "p (b c d) -> p b c d", b=NB, c=Ws)
            psl = [cacc.tile([128, CHK], F32, tag=f"pacc{c}", name=f"pacc{c}")
                   for c in range(NCHK)]
            for ci in range(4):
                wsb = cstream.tile([128, 49 * 128], BF16, tag="ws")
                nc.sync.dma_start(wsb[:, 0:T * 128], t[wname][ci])
                for tap in range(T):
                    dy, dx = divmod(tap, k)
                    lw = wsb[:, tap * 128:(tap + 1) * 128]
                    for chk in range(NCHK):
                        b, hf = divmod(chk, 2)
                        y0 = hf * 14 + 3 - pad + dy
                        x0 = 3 - pad + dx
                        nc.tensor.matmul(
                            psl[chk][:], lw,
                            f1p[ci][:, b, y0:y0 + 14, x0:x0 + 28],
                            start=(ci == 0 and tap == 0),
                            stop=(ci == 3 and tap == T - 1))
            s_col = PP_AS[ic]
            for chk in range(NCHK):
                b, hf = divmod(chk, 2)
                y0 = hf * 14
                nc.vector.tensor_scalar(
                    astg_v[:, b, ps_pad + y0:ps_pad + y0 + 14,
                           ps_pad:ps_pad + 28], psl[chk][:],
                    pp[:, s_col:s_col + 1], pp[:, s_col + 1:s_col + 2],
                    ALU.mult, ALU.add)
            emit_ag(f"aga{ic}", astg[:, 0:NB * Ws * Ws])
        if dbg_emit("aconv", []):
            return

        # ---- b-convs (dilated 3x3) ----
        for ic, (d, wname) in enumerate(BCONVS):
            Wp = 28 + 2 * d
            bstg = cstream.tile([128, NB * 30 * 30], BF16, tag="bstg")
            nc.sync.dma_start(bstg[:], t["zeros_in"][:, 0:NB * 30 * 30])
            bstg_v = bstg[:].rearrange("p (b c d) -> p b c d", b=NB, c=30)
            psl = [cacc.tile([128, CHK], F32, tag=f"pacc{c}", name=f"pacc{c}")
                   for c in range(NCHK)]
            for ci in range(4):
                bpad = cstream.tile([128, NB * 42 * 42], BF16, tag="bpad")
                bp_v = bpad[:, 0:NB * Wp * Wp].rearrange(
                    "p (b c d) -> p b c d", b=NB, c=Wp)
                nc.sync.dma_start(bpad[:, 0:NB * Wp * Wp],
                                  ag_rows(f"aga{ic}", ci))
                wsb = cstream.tile([128, 49 * 128], BF16, tag="ws")
                nc.sync.dma_start(wsb[:, 0:9 * 128], t[wname][ci])
                for tap in range(9):
                    ty, tx = divmod(tap, 3)
                    lw = wsb[:, tap * 128:(tap + 1) * 128]
                    for chk in range(NCHK):
                        b, hf = divmod(chk, 2)
                        nc.tensor.matmul(
                            psl[chk][:], lw,
                            bp_v[:, b, hf * 14 + ty * d:hf * 14 + ty * d + 14,
                                 tx * d:tx * d + 28],
                            start=(ci == 0 and tap == 0),
                            stop=(ci == 3 and tap == 8))
            s_col = PP_BS[ic]
            for chk in range(NCHK):
                b, hf = divmod(chk, 2)
                y0 = hf * 14
                nc.vector.tensor_scalar(
                    bstg_v[:, b, 1 + y0:15 + y0, 1:29], psl[chk][:],
                    pp[:, s_col:s_col + 1], pp[:, s_col + 1:s_col + 2],
                    ALU.mult, ALU.add)
            emit_ag(f"agb{ic}", bstg[:])
        if dbg_emit("bconv", []):
            return

        # ---- cat conv (3x3, 1536->128) + res (1x1) -> gelu -> pbn ----
        g_sb = cv1.tile([128, NB, N], BF16, tag="g")
        pcat = [cacc.tile([128, CHK], F32, tag=f"pacc{c}", name=f"pacc{c}") for c in range(NCHK)]
        for ci in range(12):
            jj, cbb = divmod(ci, 4)
            catw = cstream.tile([128, NB, 30, 30], BF16, tag="catw")
            nc.sync.dma_start(catw[:].rearrange("p b c d -> p (b c d)"),
                              ag_rows(f"agb{jj}", cbb))
            wsb = cstream.tile([128, 49 * 128], BF16, tag="ws")
            nc.sync.dma_start(wsb[:, 0:9 * 128], t["wcat_in"][ci])
            for tap in range(9):
                ty, tx = divmod(tap, 3)
                lw = wsb[:, tap * 128:(tap + 1) * 128]
                for chk in range(NCHK):
                    b, hf = divmod(chk, 2)
                    nc.tensor.matmul(
                        pcat[chk][:], lw,
                        catw[:, b, hf * 14 + ty:hf * 14 + ty + 14, tx:tx + 28],
                        start=(ci == 0 and tap == 0),
                        stop=(ci == 11 and tap == 8))
        wres_sb = cv1.tile([128, 4, 128], BF16, tag="wres")
        for ci in range(4):
            nc.sync.dma_start(wres_sb[:, ci, :], t["wres_in"][ci])
        for chk in range(NCHK):
            b, hf = divmod(chk, 2)
            y0 = hf * 14
            pres = cmisc.tile([128, CHK], F32, tag="pmisc")
            for ci in range(4):
                nc.tensor.matmul(pres[:], wres_sb[:, ci, :],
                                 f1p[ci][:, b, 3 + y0:17 + y0, 3:31],
                                 start=(ci == 0), stop=(ci == 3))
            rt = ctmp.tile([128, CHK], F32, tag="rt")
            nc.vector.tensor_scalar(rt[:], pres[:], pp[:, PP_RESS:PP_RESS + 1],
                                    None, ALU.mult)
            ct = ctmp.tile([128, CHK], F32, tag="ct")
            nc.vector.tensor_scalar(ct[:], pcat[chk][:],
                                    pp[:, PP_CATS:PP_CATS + 1], None, ALU.mult)
            nc.vector.tensor_tensor(ct[:], ct[:], rt[:], ALU.add)
            gg = ctmp.tile([128, CHK], F32, tag="gg")
            nc.scalar.activation(gg[:], ct[:], GELU,
                                 bias=pp[:, PP_CATRT:PP_CATRT + 1])
            nc.vector.tensor_scalar(g_sb[:, b, y0 * 28:y0 * 28 + CHK], gg[:],
                                    pp[:, PP_PBNS:PP_PBNS + 1],
                                    pp[:, PP_PBNT:PP_PBNT + 1],
                                    ALU.mult, ALU.add)
        emit_ag("agg", g_sb[:].rearrange("p b n -> p (b n)"))
        if dbg_emit("gconv", [g_sb[:, 0, :]]):
            return

        # ---- p2 (1x1, 512->64) + BN ----
        gf = cv1.tile([128, 4, NB * N], BF16, tag="gf")
        for ci in range(4):
            nc.sync.dma_start(gf[:, ci, :], ag_rows("agg", ci))
        wp2_sb = cv1.tile([128, 4, 64], BF16, tag="wp2")
        for ci in range(4):
            nc.sync.dma_start(wp2_sb[:, ci, :], t["wp2_in"][ci])
        mlpT = cv1.tile([64, NB, N], F32, tag="mlpT")
        for chk in range(NCHK):
            b, hf = divmod(chk, 2)
            pp2 = cmisc.tile([128, CHK], F32, tag="pmisc")
            for ci in range(4):
                nc.tensor.matmul(
                    pp2[0:64, :], wp2_sb[:, ci, :],
                    gf[:, ci, b * N + hf * CHK: b * N + (hf + 1) * CHK],
                    start=(ci == 0), stop=(ci == 3))
            nc.vector.tensor_scalar(mlpT[:, b, hf * CHK:(hf + 1) * CHK],
                                    pp2[0:64, :],
                                    pp[0:64, PP_P2S:PP_P2S + 1],
                                    pp[0:64, PP_P2T:PP_P2T + 1],
                                    ALU.mult, ALU.add)
        if dbg_emit("p2", [mlpT[:, 0, :], mlpT[:, 1, :]]):
            return

        # ---- residual add (x1 slice via transpose) into dw padded buffer ----
        dwp = cv1.tile([64, NB, 30, 30], F32, tag="dwp")
        nc.sync.dma_start(dwp[:].rearrange("p a c d -> p (a c d)"),
                          t["zeros_in"].ap().bitcast(F32)[0:64, 0:NB * 30 * 30])
        for b2 in range(NB):
            for j in range(NTT):
                pt = cmisc.tile([64, PTOK], F32, tag="pmisc")
                nc.tensor.transpose(pt[:], x1loc[:, b2 * NTT + j, :],
                                    ident_f[0:PTOK, 0:PTOK])
                nc.vector.tensor_tensor(
                    dwp[:, b2, 1 + 4 * j:5 + 4 * j, 1:29],
                    pt[:].rearrange("p (a b) -> p a b", a=4),
                    mlpT[:, b2, j * PTOK:(j + 1) * PTOK].rearrange(
                        "p (a b) -> p a b", a=4),
                    ALU.add)

        # ---- depthwise 3x3 + bias + residual ----
        acc = cv1.tile([64, NB, 28, 28], F32, tag="acc")
        nc.vector.tensor_scalar(acc[:], dwp[:, :, 1:29, 1:29],
                                pp[0:64, PP_DWB:PP_DWB + 1], None, ALU.add)
        for tap in range(9):
            ty, tx = divmod(tap, 3)
            tmp = ctmp.tile([64, NB, 28, 28], F32, tag="dwt")
            nc.scalar.activation(tmp[:], dwp[:, :, ty:ty + 28, tx:tx + 28],
                                 AF.Copy,
                                 scale=pp[0:64, PP_DWW + tap:PP_DWW + tap + 1])
            nc.vector.tensor_tensor(acc[:], acc[:], tmp[:], ALU.add)

        # ---- transpose back to token-major and write out ----
        for b2 in range(NB):
            for j in range(NTT):
                pf = cmisc.tile([PTOK, 64], F32, tag="pmisc")
                nc.tensor.transpose(
                    pf[:],
                    acc[:, b2, :, :].rearrange(
                        "p c d -> p (c d)")[:, j * PTOK:(j + 1) * PTOK],
                    ident_f[0:64, 0:64])
                ot = ctmp.tile([PTOK, 64], F32, tag="ot")
                nc.vector.tensor_copy(ot[:], pf[:])
                nc.sync.dma_start(t["out_d"][b2, j * PTOK:(j + 1) * PTOK, :],
                                  ot[:])




# =========================================================================
# Host side
# =========================================================================
_CACHE = {}


def _get_program(reps=1):
    key = ("prog", DEBUG_STAGE, reps, str(GELU))
    if key not in _CACHE:
        _CACHE[key] = _build(DEBUG_STAGE, reps)
    return _CACHE[key]


def _bf(a):
    return np.ascontiguousarray(np.asarray(a, np.float32).astype(
        ml_dtypes.bfloat16))


def _bn_fold(bn):
    s = np.asarray(bn["g"], np.float32) / np.sqrt(
        np.asarray(bn["v"], np.float32) + EPS)
    tt = np.asarray(bn["b"], np.float32) - np.asarray(bn["m"], np.float32) * s
    return s, tt


def _conv_lhst(w, cb, nci, ncout=128):
    """w [Cout, Cin, k, k] -> [nci, 128, k*k*ncout] bf16 (lhsT layout)."""
    w = np.asarray(w, np.float32)
    k = w.shape[2]
    ws = w[cb * ncout:(cb + 1) * ncout]          # [ncout, Cin, k, k]
    ws = ws.transpose(1, 2, 3, 0)                # [Cin, k, k, ncout]
    ws = ws.reshape(nci, 128, k * k * ncout)
    return _bf(ws)


def _prep_core(core, x, rp, p):
    bh, cb = core // 4, core % 4
    m = {}
    m["x_in"] = np.ascontiguousarray(np.asarray(x, np.float32))
    m["rpT_in"] = _bf(np.asarray(rp[core], np.float32).T)
    m["bsel_in"] = np.array([[2 * bh]], np.uint32)
    m["csel_in"] = np.array([[64 * cb]], np.uint32)

    g1 = np.asarray(p["ln1_g"], np.float32)
    b1 = np.asarray(p["ln1_b"], np.float32)
    hs = slice(core * HD, (core + 1) * HD)
    sc = HD ** -0.5
    Wq = np.asarray(p["Wq"], np.float32)
    Wk = np.asarray(p["Wk"], np.float32)
    Wv = np.asarray(p["Wv"], np.float32)
    m["wq_in"] = _bf(((g1[:, None] * Wq)[:, hs] * sc).reshape(2, 128, HD))
    m["wk_in"] = _bf((g1[:, None] * Wk)[:, hs].reshape(2, 128, HD))
    m["wv_in"] = _bf((g1[:, None] * Wv)[:, hs].reshape(2, 128, HD))
    m["wp_in"] = _bf(np.asarray(p["Wp"], np.float32).reshape(2, 128, C))
    # q/k biases go to pp; v bias broadcast
    bq = (b1 @ Wq)[hs] * sc
    bk = (b1 @ Wk)[hs]
    bv = (b1 @ Wv)[hs]
    m["bv_in"] = np.ascontiguousarray(bv.reshape(1, HD).astype(np.float32))
    m["bp_in"] = np.ascontiguousarray(
        np.asarray(p["bp"], np.float32).reshape(1, C))
    saw = np.asarray(p["sa_w"], np.float32)[0]   # [2, 7, 7]
    tz = np.zeros((7, 68, 28), np.float32)
    for dxx in range(7):
        for cch in range(2):
            for yout in range(28):
                for dyy in range(7):
                    tz[dxx, cch * 34 + yout + dyy, yout] = saw[cch, dyy, dxx]
    m["tz_in"] = tz

    m["wc1_in"] = _conv_lhst(p["c1_w"], cb, 2)
    for ic, (k, pad, name) in enumerate(ACONVS):
        key = {0: "b1a_w", 1: "b2a_w", 2: "b3a_w"}[ic]
        m[name] = _conv_lhst(p[key], cb, 4)
    for ic, (d, name) in enumerate(BCONVS):
        key = {0: "b1b_w", 1: "b2b_w", 2: "b3b_w"}[ic]
        m[name] = _conv_lhst(p[key], cb, 4)
    # cat conv, input channels in AG order: block ci=(cbb*3+jj) holds
    # original channels jj*512 + cbb*128 + [0..128)
    wcat = np.asarray(p["cat_w"], np.float32)[cb * 128:(cb + 1) * 128]
    blocks = []
    for jj in range(3):
        for cbb in range(4):
            chs = wcat[:, jj * 512 + cbb * 128: jj * 512 + (cbb + 1) * 128]
            blocks.append(chs.transpose(1, 2, 3, 0).reshape(128, 9 * 128))
    m["wcat_in"] = _bf(np.stack(blocks))
    m["wres_in"] = _conv_lhst(p["res_w"], cb, 4)
    wp2 = np.asarray(p["p2_w"], np.float32)[cb * 64:(cb + 1) * 64, :, 0, 0]
    m["wp2_in"] = _bf(wp2.T.reshape(4, 128, 64))

    # pp table
    pp = np.zeros((128, NPP), np.float32)
    pp[:HD, PP_QB] = bq
    pp[:HD, PP_KB] = bk
    cs = slice(cb * 128, (cb + 1) * 128)
    pp[:, PP_C1B] = np.asarray(p["c1_b"], np.float32)[cs]
    s, tt = _bn_fold(p["c1_bn"])
    pp[:, PP_C1S], pp[:, PP_C1T] = s[cs], tt[cs]
    for ic, key in ((0, "b1a_bn"), (1, "b2a_bn"), (2, "b3a_bn")):
        s, tt = _bn_fold(p[key])
        pp[:, PP_AS[ic]], pp[:, PP_AS[ic] + 1] = s[cs], tt[cs]
    for ic, key in ((0, "b1b_bn"), (1, "b2b_bn"), (2, "b3b_bn")):
        s, tt = _bn_fold(p[key])
        pp[:, PP_BS[ic]], pp[:, PP_BS[ic] + 1] = s[cs], tt[cs]
    s_cat, t_cat = _bn_fold(p["cat_bn"])
    s_res, t_res = _bn_fold(p["res_bn"])
    pp[:, PP_CATS], pp[:, PP_RESS] = s_cat[cs], s_res[cs]
    pp[:, PP_CATRT] = (t_cat + t_res)[cs]
    s, tt = _bn_fold(p["pbn_bn"])
    pp[:, PP_PBNS], pp[:, PP_PBNT] = s[cs], tt[cs]
    s, tt = _bn_fold(p["p2_bn"])
    c64 = slice(cb * 64, (cb + 1) * 64)
    for b2 in range(2):
        pp[b2 * 64:(b2 + 1) * 64, PP_P2S] = s[c64]
        pp[b2 * 64:(b2 + 1) * 64, PP_P2T] = \
            (tt + np.asarray(p["p2_b"], np.float32) * s)[c64]
    dwb = np.asarray(p["dw_b"], np.float32)[c64]
    dww = np.asarray(p["dw_w"], np.float32)[c64, 0]   # [64, 3, 3]
    for b2 in range(2):
        pp[b2 * 64:(b2 + 1) * 64, PP_DWB] = dwb
        for tap in range(9):
            pp[b2 * 64:(b2 + 1) * 64, PP_DWW + tap] = dww[:, tap // 3, tap % 3]
    g2 = np.asarray(p["ln2_g"], np.float32)
    b2v = np.asarray(p["ln2_b"], np.float32)
    for c2 in range(2):
        pp[:, PP_G2[c2]] = g2[c2 * 128:(c2 + 1) * 128]
        pp[:, PP_B2[c2]] = b2v[c2 * 128:(c2 + 1) * 128]
    m["pp_in"] = pp
    m["zeros_in"] = np.zeros((128, 3600), ml_dtypes.bfloat16)
    return m


def kernel(x, relative_pos, params, H, W, _trace=False):
    assert int(H) == Hh and int(W) == Ww
    nc = _get_program()
    in_maps = [_prep_core(c, x, relative_pos, params) for c in range(NCORES)]
    res = bass_utils.run_bass_kernel_spmd(
        nc, in_maps, core_ids=list(range(NCORES)), trace=_trace)
    if DEBUG_STAGE is not None:
        kernel.last_dbg = [r["dbg"] for r in res.results]
    kernel.last_exec_ns = res.exec_time_ns
    kernel.last_res = res
    out = np.empty((B, N, C), np.float32)
    for core in range(NCORES):
        bh, cb = core // 4, core % 4
        out[2 * bh:2 * bh + 2, :, 64 * cb:64 * cb + 64] = \
            res.results[core]["out"]
    return out
